# revision 12
# baseline (speedup 1.0000x reference)
"""Distributed Trainium2 (Bass/Tile) kernel for AdaptiveGCNLayer.

Reference semantics (N=4096 nodes, C=512 channels):
    adj   = x @ W_adj @ x.T + I                      [N, N]
    adj   = d^-1/2 * adj * d^-1/2   (row sums d)     -- values then DISCARDED:
    A     = (adj != 0) with forced unit diagonal     (dense_to_sparse keeps only
                                                      the nonzero pattern)
    deg   = A.sum(1); dis = deg^-1/2 (0 if deg<=0)
    out   = (dis[:,None] * A * dis[None,:]) @ (x @ W_gcn) + b

Scaling rows/cols by nonzero (or NaN/inf) factors never changes the !=0
pattern, so A == (x @ W_adj @ x.T != 0) except on the measure-zero event of
an exactly-zero f32 entry; the first normalization is therefore not
materialized, and the adjacency can be computed at any precision (fp8 here)
since only its zero pattern survives.

Sharding (8 cores, 1-D node partition, R=512 rows each): core i computes its
adjacency block in TRANSPOSED layout adjT [N, R] (directly usable as the
stationary operand of the final aggregation), masks it to {0,1} bf16, and
reduces mask -> deg for its rows (ones-matmul chain on the TensorEngine).

Collective structure: this environment has a ~40-46us rank-dispatch skew —
the mesh for ANY collective begins only once the LAST core reaches its
trigger, so the measured core-0 span is
  skew + slowest-core time-to-trigger + mesh + post-collective work.
There is exactly ONE collective: the 16KB deg AllGather (~8us mesh).
y = x @ W_gcn is computed locally IN FULL on every core (~30us of
redundant TensorE work) inside the otherwise-dead window while waiting
for the deg exchange — redundant FLOPs are free there, wire time is not.
Critical path: loads -> xwT -> adjacency+mask+deg (~30us local) -> deg
AllGather -> dis -> aggregation.

Scheduling notes (hard-won from traces):
  - ~7.3us fixed engine-bringup preamble before any user work
  - each DMA ring delivers ~100GB/s serialized in ring order, so ring
    POSITION is arrival time; the critical bytes (1a operands, then xT8)
    sit at the FRONT of the sync+scalar rings, split between them
  - xT8 is loaded in half-column chunks so adjacency tiles 0-15 can
    start ~2.5us before the full tensor lands
  - the Tile scheduler reorders per-engine instructions by readiness, so
    program order alone cannot keep y-phase DVE/ACT work (casts) out of
    the mask window; instead xTbf/gcnW are placed ring-LATE so y matmuls
    physically cannot start before the deg trigger has fired
  - gpsimd's ring carries ONLY the deg bounce + AllGather + readbacks
    (gpsimd-ring completion signaling is slow; nothing critical rides it)
  - mask computation is split DVE(not_equal) 2 : ACT(sign^2) 1, keeping
    both engines under the phase-2 wall; adjacency runs fp8 DoubleRow
  - the deg payload is written row-linear; readbacks pay the strided
    transpose (16KB, ~1us)
  - the bias enters through a rank-1 matmul sqrt(deg_r) (x) bias folded
    into the aggregation PSUM (cancels the later dis_r row scaling)
"""

import numpy as np

from concourse import bacc, mybir, tile
from concourse.bass_utils import run_bass_kernel_spmd

N_CORES = 8
N = 4096               # nodes
C = 512                # channels (C_IN == C_OUT)
R = N // N_CORES       # 512 rows per core
P = 128                # SBUF partitions
KT = C // P            # 4 contraction tiles
NT = N // P            # 32 node tiles
MT = R // P            # 4 row tiles per core
H = N // 2             # xT8 half-chunk columns

F32 = mybir.dt.float32
BF16 = mybir.dt.bfloat16
F8 = mybir.dt.float8e4
BF = mybir.dt.np(BF16)
F8NP = mybir.dt.np(F8)
DR = mybir.MatmulPerfMode.DoubleRow

_cache = {}


def _build():
    nc = bacc.Bacc("TRN2", target_bir_lowering=False, debug=False,
                   num_devices=N_CORES)

    xT8 = nc.dram_tensor("xT8", [C, N], F8, kind="ExternalInput")      # x^T, full
    xTs8 = nc.dram_tensor("xTs8", [C, R], F8, kind="ExternalInput")    # own cols
    adjW8 = nc.dram_tensor("adjW8", [C, C], F8, kind="ExternalInput")
    xTbf = nc.dram_tensor("xTbf", [C, N], BF16, kind="ExternalInput")  # x^T, full
    gcnW = nc.dram_tensor("gcnW", [C, C], BF16, kind="ExternalInput")
    bias = nc.dram_tensor("bias", [1, C], BF16, kind="ExternalInput")
    out = nc.dram_tensor("out", [R, C], F32, kind="ExternalOutput")

    rg = [list(range(N_CORES))]

    with tile.TileContext(nc) as tc:
        with (
            tc.tile_pool(name="sb", bufs=1) as sb,
            tc.tile_pool(name="sbo", bufs=2) as sbo,
            tc.tile_pool(name="dram", bufs=1, space="DRAM") as dram,
            tc.tile_pool(name="ps_a", bufs=3, space="PSUM") as ps_a,
            tc.tile_pool(name="ps_adj", bufs=2, space="PSUM") as ps_adj,
            tc.tile_pool(name="ps_deg", bufs=1, space="PSUM") as ps_deg,
            tc.tile_pool(name="ps_fin", bufs=2, space="PSUM") as ps_fin,
        ):
            # ---- SBUF tiles -------------------------------------------------
            adjW8_sb = sb.tile([P, KT, C], F8, name="adjW8_sb", tag="adjW8_sb")
            xTs8_sb = sb.tile([P, KT, R], F8, name="xTs8_sb", tag="xTs8_sb")
            xT8_sb = sb.tile([P, KT, N], F8, name="xT8_sb", tag="xT8_sb")
            xTbf_sb = [sb.tile([P, N], BF16, name=f"xTbf{k}", tag=f"xTbf{k}") for k in range(KT)]
            gcnW_sb = [sb.tile([P, C], BF16, name=f"gcnW{k}", tag=f"gcnW{k}") for k in range(KT)]
            bias_sb = sb.tile([1, C], BF16, name="bias_sb", tag="bias_sb")
            ones_col = sb.tile([P, 1], BF16, name="ones_col", tag="ones_col")
            ones_f32 = sb.tile([P, 1], F32, name="ones_f32", tag="ones_f32")
            scr = sb.tile([1, 8], F32, name="scr", tag="scr")

            # ---- input loads: ring position == arrival time ---------------
            # The ACT engine's compute shares its sequencer with the scalar
            # DMA ring, so the scalar ring carries ONLY the critical bytes
            # (1a operands + xT8) plus two ring-late xTbf slices; everything
            # else rides sync.
            nc.sync.dma_start(adjW8_sb[:, 0, :], adjW8[0:P, :])
            nc.sync.dma_start(adjW8_sb[:, 1, :], adjW8[P:2 * P, :])
            nc.sync.dma_start(xTs8_sb[:, 0, :], xTs8[0:P, :])
            nc.sync.dma_start(xTs8_sb[:, 1, :], xTs8[P:2 * P, :])
            nc.scalar.dma_start(adjW8_sb[:, 2, :], adjW8[2 * P:3 * P, :])
            nc.scalar.dma_start(adjW8_sb[:, 3, :], adjW8[3 * P:4 * P, :])
            nc.scalar.dma_start(xTs8_sb[:, 2, :], xTs8[2 * P:3 * P, :])
            nc.scalar.dma_start(xTs8_sb[:, 3, :], xTs8[3 * P:4 * P, :])
            for h in range(2):
                nc.sync.dma_start(xT8_sb[:, 0, H * h:H * (h + 1)], xT8[0:P, H * h:H * (h + 1)])
                nc.sync.dma_start(xT8_sb[:, 1, H * h:H * (h + 1)], xT8[P:2 * P, H * h:H * (h + 1)])
                nc.scalar.dma_start(xT8_sb[:, 2, H * h:H * (h + 1)], xT8[2 * P:3 * P, H * h:H * (h + 1)])
                nc.scalar.dma_start(xT8_sb[:, 3, H * h:H * (h + 1)], xT8[3 * P:4 * P, H * h:H * (h + 1)])
            nc.sync.dma_start(bias_sb[:, :], bias[:, :])
            for k in range(KT):
                nc.sync.dma_start(gcnW_sb[k][:, :], gcnW[P * k:P * (k + 1), :])
            # y stationary ring-LATE: y matmuls start only as these land, so
            # their casts cannot crowd the mask window; the y accumulation
            # consumes the k-slices in arrival order (1,0,3,2 across rings)
            nc.sync.dma_start(xTbf_sb[0][:, :], xTbf[0:P, :])
            nc.scalar.dma_start(xTbf_sb[1][:, :], xTbf[P:2 * P, :])
            nc.sync.dma_start(xTbf_sb[2][:, :], xTbf[2 * P:3 * P, :])
            nc.scalar.dma_start(xTbf_sb[3][:, :], xTbf[3 * P:4 * P, :])

            nc.vector.memset(ones_col[:, :], 1.0)
            nc.vector.memset(ones_f32[:, :], 1.0)
            # preload DVE reciprocal / ACT sqrt+sign lookup tables off the
            # critical path (first use otherwise costs ~1.3us each)
            nc.vector.memset(scr[:, 0:4], 4.0)
            nc.vector.reciprocal(scr[:, 4:8], scr[:, 0:4])
            nc.scalar.sqrt(scr[:, 4:8], scr[:, 0:4])
            nc.scalar.sign(scr[:, 4:8], scr[:, 0:4])

            # ---- phase 1a: xwT[j, r] = sum_c W_adj[c, j] x^T[c, r]  (fp8 DR)
            xwT8_sb = sb.tile([P, KT, R], F8, name="xwT8_sb", tag="xwT8_sb")
            for j in range(KT):
                pa = ps_a.tile([P, R], F32, name=f"psa{j}", tag="psa")
                for k in range(0, KT, 2):
                    nc.tensor.matmul(pa[:, :],
                                     adjW8_sb[:, k:k + 2, P * j:P * (j + 1)],
                                     xTs8_sb[:, k:k + 2, :],
                                     start=(k == 0), stop=(k == KT - 2),
                                     perf_mode=DR)
                nc.vector.tensor_copy(xwT8_sb[:, j, :], pa[:, :])

            # ---- phase 2: adjT tiles (fp8 DR), mask (bf16), deg ------------
            mask_sb = [sb.tile([P, R], BF16, name=f"mask{t}", tag=f"mask{t}") for t in range(NT)]
            pdeg = ps_deg.tile([1, R], F32, name="pdeg", tag="pdeg")
            for t in range(NT):
                pt = ps_adj.tile([P, R], F32, name=f"psadj{t}", tag="psadj")
                for k in range(0, KT, 2):
                    nc.tensor.matmul(pt[:, :],
                                     xT8_sb[:, k:k + 2, P * t:P * (t + 1)],
                                     xwT8_sb[:, k:k + 2, :],
                                     start=(k == 0), stop=(k == KT - 2),
                                     perf_mode=DR)
                # mask split DVE (not_equal) 2 : ACT (sign^2) 1
                if t % 3 == 2:
                    nc.scalar.sign(mask_sb[t][:, :], pt[:, :])
                    nc.scalar.square(mask_sb[t][:, :], mask_sb[t][:, :])
                else:
                    nc.vector.tensor_scalar(mask_sb[t][:, :], pt[:, :], 0.0, None,
                                            mybir.AluOpType.not_equal)
                nc.tensor.matmul(pdeg[:, :], ones_col[:, :], mask_sb[t][:, :],
                                 start=(t == 0), stop=(t == NT - 1))

            deg_own = sb.tile([1, R], F32, name="deg_own", tag="deg_own")
            nc.vector.tensor_copy(deg_own[:, :], pdeg[:, :])
            # sqrt(deg) row-vector: cancels the dis_r row scaling for the bias.
            invdis_row = sb.tile([1, R], BF16, name="invdis_row", tag="invdis_row")
            nc.scalar.sqrt(invdis_row[:, :], deg_own[:, :])

            # ---- the ONE collective: AllGather deg (16KB) ------------------
            degb_in = dram.tile([R], F32, name="degb_in", tag="degb_in")
            degb_out = dram.tile([N], F32, addr_space="Shared", name="degb_out", tag="degb_out")
            # row-linear payload: degb_in[r] = deg(own row r); contiguous write
            nc.gpsimd.dma_start(degb_in.rearrange("(m p) -> m p", p=P), deg_own[:, :])
            nc.gpsimd.collective_compute(
                "AllGather", mybir.AluOpType.bypass, replica_groups=rg,
                ins=[degb_in.opt()], outs=[degb_out.opt()])

            # readbacks ride SYNC (clear by then, and its completion
            # signaling is ~5us faster than gpsimd's)
            deg_ownp = sb.tile([P, MT], F32, name="deg_ownp", tag="deg_ownp")
            nc.sync.dma_start(deg_ownp[:, :], degb_in.rearrange("(m p) -> p m", p=P))
            deg_glob = sb.tile([P, NT], F32, name="deg_glob", tag="deg_glob")
            nc.sync.dma_start(
                deg_glob[:, :].rearrange("p (i m) -> p i m", i=N_CORES),
                degb_out.rearrange("(i m p) -> p i m", i=N_CORES, p=P))

            dis_own = sb.tile([P, MT], F32, name="dis_own", tag="dis_own")
            nc.vector.reciprocal(dis_own[:, :], deg_ownp[:, :])
            nc.scalar.sqrt(dis_own[:, :], dis_own[:, :])

            # ---- full y = x @ W_gcn for ALL nodes (local, no collective) ---
            # k-order (1,0,3,2) matches the xTbf arrival order across rings
            YK = (1, 0, 3, 2)
            y_sb = [sb.tile([P, C], BF16, name=f"y{t}", tag=f"y{t}") for t in range(NT)]
            for t in range(NT):
                pa = ps_a.tile([P, C], F32, name=f"psy{t}", tag="psa")
                for ki, k in enumerate(YK):
                    nc.tensor.matmul(pa[:, :],
                                     xTbf_sb[k][:, P * t:P * (t + 1)],
                                     gcnW_sb[k][:, :],
                                     start=(ki == 0), stop=(ki == KT - 1))
                if t % 3 == 2:
                    nc.scalar.copy(y_sb[t][:, :], pa[:, :])
                else:
                    nc.vector.tensor_copy(y_sb[t][:, :], pa[:, :])

            # dis = deg^-1/2 (global, post-AllGather)
            dis_glob = sb.tile([P, NT], F32, name="dis_glob", tag="dis_glob")
            nc.vector.reciprocal(dis_glob[:, :], deg_glob[:, :])
            nc.scalar.sqrt(dis_glob[:, :], dis_glob[:, :])

            # tensor warm-up gated on the gathered deg: ramps the PE p-state
            # during the dis/scale latency window so phase 3 starts hot
            psw = ps_deg.tile([1, R], F32, name="psw", tag="pdeg")
            nc.tensor.matmul(psw[:, 0:NT], ones_f32[:, :], deg_glob[:, :],
                             start=True, stop=True)
            for w in range(6):
                nc.tensor.matmul(psw[:, :], ones_col[:, :], mask_sb[NT - 1][:, :],
                                 start=True, stop=True)

            # ---- phase 3: y *= dis; out_rows = dis_r * (A @ y) + b ----------
            for t in range(NT):
                if t % 3 == 2:
                    nc.scalar.mul(y_sb[t][:, :], y_sb[t][:, :], dis_glob[:, t:t + 1])
                else:
                    nc.vector.tensor_scalar(y_sb[t][:, :], y_sb[t][:, :],
                                            dis_glob[:, t:t + 1], None,
                                            mybir.AluOpType.mult)

            # m-outer: each PSUM bank accumulates a long 33-matmul chain
            for m in range(MT):
                pf = ps_fin.tile([P, C], F32, name=f"psf{m}", tag="psf")
                for t in range(NT):
                    nc.tensor.matmul(pf[:, :],
                                     mask_sb[t][:, P * m:P * (m + 1)],
                                     y_sb[t][:, :],
                                     start=(t == 0), stop=False)
                # += sqrt(deg_r) (x) bias  — cancels against the dis_r scaling
                nc.tensor.matmul(pf[:, :],
                                 invdis_row[:, P * m:P * (m + 1)],
                                 bias_sb[:, :],
                                 start=False, stop=True)
                ot = sbo.tile([P, C], F32, name=f"outt{m}", tag="outt")
                nc.vector.tensor_scalar(ot[:, :], pf[:, :], dis_own[:, m:m + 1],
                                        None, mybir.AluOpType.mult)
                eng = nc.sync if m % 2 == 0 else nc.scalar
                eng.dma_start(out[P * m:P * (m + 1), :], ot[:, :])

    nc.compile()
    return nc


def _get_nc():
    if "nc" not in _cache:
        _cache["nc"] = _build()
    return _cache["nc"]


def _run(inputs, trace=False, trace_cores=None):
    x = np.asarray(inputs["x"], dtype=np.float32)
    adj_weight = np.asarray(inputs["adj_weight"], dtype=np.float32)
    gcn_weight = np.asarray(inputs["gcn_weight"], dtype=np.float32)
    gcn_bias = np.asarray(inputs["gcn_bias"], dtype=np.float32)

    xT = np.ascontiguousarray(x.T)                     # [C, N] f32
    xT8 = xT.astype(F8NP)
    xTbf = xT.astype(BF)
    adjW8 = adj_weight.astype(F8NP)
    gcnW = gcn_weight.astype(BF)
    bias_bf = gcn_bias.reshape(1, C).astype(BF)

    in_maps = []
    for i in range(N_CORES):
        in_maps.append({
            "xT8": xT8,
            "xTs8": np.ascontiguousarray(xT8[:, R * i:R * (i + 1)]),
            "adjW8": adjW8,
            "xTbf": xTbf,
            "gcnW": gcnW,
            "bias": bias_bf,
        })

    nc = _get_nc()
    res = run_bass_kernel_spmd(nc, in_maps, core_ids=list(range(N_CORES)),
                               trace=trace, trace_cores=trace_cores)
    full = np.concatenate([res.results[i]["out"] for i in range(N_CORES)], axis=0)
    return full, res


def kernel(**inputs):
    full, _ = _run(inputs, trace=False)
    return full


# revision 14
# speedup vs baseline: 1.0207x; 1.0207x over previous
"""Distributed Trainium2 (Bass/Tile) kernel for AdaptiveGCNLayer.

Reference semantics (N=4096 nodes, C=512 channels):
    adj   = x @ W_adj @ x.T + I                      [N, N]
    adj   = d^-1/2 * adj * d^-1/2   (row sums d)     -- values then DISCARDED:
    A     = (adj != 0) with forced unit diagonal     (dense_to_sparse keeps only
                                                      the nonzero pattern)
    deg   = A.sum(1); dis = deg^-1/2 (0 if deg<=0)
    out   = (dis[:,None] * A * dis[None,:]) @ (x @ W_gcn) + b

Scaling rows/cols by nonzero (or NaN/inf) factors never changes the !=0
pattern, so A == (x @ W_adj @ x.T != 0) except on the measure-zero event of
an exactly-zero f32 entry; the first normalization is therefore not
materialized, and the adjacency can be computed at any precision (fp8 here)
since only its zero pattern survives.

Sharding (8 cores, 1-D node partition, R=512 rows each): core i computes its
adjacency block in TRANSPOSED layout adjT [N, R] (directly usable as the
stationary operand of the final aggregation), masks it to {0,1} bf16, and
reduces mask -> deg for its rows (ones-matmul chain on the TensorEngine).

Collective structure: this environment has a ~40-46us rank-dispatch skew —
the mesh for ANY collective begins only once the LAST core reaches its
trigger, so the measured core-0 span is
  skew + slowest-core time-to-trigger + mesh + post-collective work.
There is exactly ONE collective: the 16KB deg AllGather (~8us mesh).
y = x @ W_gcn is computed locally IN FULL on every core (~30us of
redundant TensorE work) inside the otherwise-dead window while waiting
for the deg exchange — redundant FLOPs are free there, wire time is not.
Critical path: loads -> xwT -> adjacency+mask+deg (~30us local) -> deg
AllGather -> dis -> aggregation.

Scheduling notes (hard-won from traces):
  - ~7.3us fixed engine-bringup preamble before any user work
  - each DMA ring delivers ~100GB/s serialized in ring order, so ring
    POSITION is arrival time; the critical bytes (1a operands, then xT8)
    sit at the FRONT of the sync+scalar rings, split between them
  - xT8 is loaded in half-column chunks so adjacency tiles 0-15 can
    start ~2.5us before the full tensor lands
  - the Tile scheduler reorders per-engine instructions by readiness, so
    program order alone cannot keep y-phase DVE/ACT work (casts) out of
    the mask window; instead xTbf/gcnW are placed ring-LATE so y matmuls
    physically cannot start before the deg trigger has fired
  - gpsimd's ring carries ONLY the deg bounce + AllGather + readbacks
    (gpsimd-ring completion signaling is slow; nothing critical rides it)
  - mask computation is split DVE(not_equal) 2 : ACT(sign^2) 1, keeping
    both engines under the phase-2 wall; adjacency runs fp8 DoubleRow
  - the deg payload is written row-linear; readbacks pay the strided
    transpose (16KB, ~1us)
  - the bias enters through a rank-1 matmul sqrt(deg_r) (x) bias folded
    into the aggregation PSUM (cancels the later dis_r row scaling)
"""

import numpy as np

from concourse import bacc, mybir, tile
from concourse.bass_utils import run_bass_kernel_spmd

N_CORES = 8
N = 4096               # nodes
C = 512                # channels (C_IN == C_OUT)
R = N // N_CORES       # 512 rows per core
P = 128                # SBUF partitions
KT = C // P            # 4 contraction tiles
NT = N // P            # 32 node tiles
MT = R // P            # 4 row tiles per core
H = N // 2             # xT8 half-chunk columns

F32 = mybir.dt.float32
BF16 = mybir.dt.bfloat16
F8 = mybir.dt.float8e4
BF = mybir.dt.np(BF16)
F8NP = mybir.dt.np(F8)
DR = mybir.MatmulPerfMode.DoubleRow

_cache = {}


def _build():
    nc = bacc.Bacc("TRN2", target_bir_lowering=False, debug=False,
                   num_devices=N_CORES)

    xT8 = nc.dram_tensor("xT8", [C, N], F8, kind="ExternalInput")      # x^T, full
    xTs8 = nc.dram_tensor("xTs8", [C, R], F8, kind="ExternalInput")    # own cols
    adjW8 = nc.dram_tensor("adjW8", [C, C], F8, kind="ExternalInput")
    xTbf = nc.dram_tensor("xTbf", [C, N], BF16, kind="ExternalInput")  # x^T, full
    gcnW = nc.dram_tensor("gcnW", [C, C], BF16, kind="ExternalInput")
    bias = nc.dram_tensor("bias", [1, C], BF16, kind="ExternalInput")
    out = nc.dram_tensor("out", [R, C], F32, kind="ExternalOutput")

    rg = [list(range(N_CORES))]

    with tile.TileContext(nc) as tc:
        with (
            tc.tile_pool(name="sb", bufs=1) as sb,
            tc.tile_pool(name="sbo", bufs=2) as sbo,
            tc.tile_pool(name="dram", bufs=1, space="DRAM") as dram,
            tc.tile_pool(name="ps_a", bufs=3, space="PSUM") as ps_a,
            tc.tile_pool(name="ps_adj", bufs=2, space="PSUM") as ps_adj,
            tc.tile_pool(name="ps_deg", bufs=1, space="PSUM") as ps_deg,
            tc.tile_pool(name="ps_fin", bufs=2, space="PSUM") as ps_fin,
        ):
            # ---- SBUF tiles -------------------------------------------------
            adjW8_sb = sb.tile([P, KT, C], F8, name="adjW8_sb", tag="adjW8_sb")
            xTs8_sb = sb.tile([P, KT, R], F8, name="xTs8_sb", tag="xTs8_sb")
            xT8_sb = sb.tile([P, KT, N], F8, name="xT8_sb", tag="xT8_sb")
            xTbf_sb = [sb.tile([P, N], BF16, name=f"xTbf{k}", tag=f"xTbf{k}") for k in range(KT)]
            gcnW_sb = [sb.tile([P, C], BF16, name=f"gcnW{k}", tag=f"gcnW{k}") for k in range(KT)]
            bias_sb = sb.tile([1, C], BF16, name="bias_sb", tag="bias_sb")
            ones_col = sb.tile([P, 1], BF16, name="ones_col", tag="ones_col")
            ones_f32 = sb.tile([P, 1], F32, name="ones_f32", tag="ones_f32")
            scr = sb.tile([1, 8], F32, name="scr", tag="scr")

            # ---- input loads: ring position == arrival time ---------------
            # The ACT engine's compute shares its sequencer with the scalar
            # DMA ring, so the scalar ring carries ONLY the critical bytes
            # (1a operands + xT8) plus two ring-late xTbf slices; everything
            # else rides sync.
            nc.sync.dma_start(adjW8_sb[:, 0, :], adjW8[0:P, :])
            nc.sync.dma_start(adjW8_sb[:, 1, :], adjW8[P:2 * P, :])
            nc.sync.dma_start(xTs8_sb[:, 0, :], xTs8[0:P, :])
            nc.sync.dma_start(xTs8_sb[:, 1, :], xTs8[P:2 * P, :])
            nc.scalar.dma_start(adjW8_sb[:, 2, :], adjW8[2 * P:3 * P, :])
            nc.scalar.dma_start(adjW8_sb[:, 3, :], adjW8[3 * P:4 * P, :])
            nc.scalar.dma_start(xTs8_sb[:, 2, :], xTs8[2 * P:3 * P, :])
            nc.scalar.dma_start(xTs8_sb[:, 3, :], xTs8[3 * P:4 * P, :])
            for h in range(2):
                nc.sync.dma_start(xT8_sb[:, 0, H * h:H * (h + 1)], xT8[0:P, H * h:H * (h + 1)])
                nc.sync.dma_start(xT8_sb[:, 1, H * h:H * (h + 1)], xT8[P:2 * P, H * h:H * (h + 1)])
                nc.scalar.dma_start(xT8_sb[:, 2, H * h:H * (h + 1)], xT8[2 * P:3 * P, H * h:H * (h + 1)])
                nc.scalar.dma_start(xT8_sb[:, 3, H * h:H * (h + 1)], xT8[3 * P:4 * P, H * h:H * (h + 1)])
            nc.sync.dma_start(bias_sb[:, :], bias[:, :])
            for k in range(KT):
                nc.sync.dma_start(gcnW_sb[k][:, :], gcnW[P * k:P * (k + 1), :])
            # y stationary ring-LATE: y matmuls start only as these land, so
            # their casts cannot crowd the mask window; the y accumulation
            # consumes the k-slices in arrival order (1,0,3,2 across rings)
            nc.sync.dma_start(xTbf_sb[0][:, :], xTbf[0:P, :])
            nc.scalar.dma_start(xTbf_sb[1][:, :], xTbf[P:2 * P, :])
            nc.sync.dma_start(xTbf_sb[2][:, :], xTbf[2 * P:3 * P, :])
            nc.scalar.dma_start(xTbf_sb[3][:, :], xTbf[3 * P:4 * P, :])

            nc.vector.memset(ones_col[:, :], 1.0)
            nc.vector.memset(ones_f32[:, :], 1.0)
            # preload DVE reciprocal / ACT sqrt+sign lookup tables off the
            # critical path (first use otherwise costs ~1.3us each)
            nc.vector.memset(scr[:, 0:4], 4.0)
            nc.vector.reciprocal(scr[:, 4:8], scr[:, 0:4])
            nc.scalar.sqrt(scr[:, 4:8], scr[:, 0:4])
            nc.scalar.sign(scr[:, 4:8], scr[:, 0:4])

            # ---- phase 1a: xwT[j, r] = sum_c W_adj[c, j] x^T[c, r]  (fp8 DR)
            xwT8_sb = sb.tile([P, KT, R], F8, name="xwT8_sb", tag="xwT8_sb")
            for j in range(KT):
                pa = ps_a.tile([P, R], F32, name=f"psa{j}", tag="psa")
                for k in range(0, KT, 2):
                    nc.tensor.matmul(pa[:, :],
                                     adjW8_sb[:, k:k + 2, P * j:P * (j + 1)],
                                     xTs8_sb[:, k:k + 2, :],
                                     start=(k == 0), stop=(k == KT - 2),
                                     perf_mode=DR)
                if j < 2:
                    nc.vector.tensor_copy(xwT8_sb[:, j, :], pa[:, :])
                else:
                    nc.scalar.copy(xwT8_sb[:, j, :], pa[:, :])

            # ---- phase 2: adjT tiles (fp8 DR), mask (bf16), deg ------------
            mask_sb = [sb.tile([P, R], BF16, name=f"mask{t}", tag=f"mask{t}") for t in range(NT)]
            pdeg = ps_deg.tile([1, R], F32, name="pdeg", tag="pdeg")
            for t in range(NT):
                pt = ps_adj.tile([P, R], F32, name=f"psadj{t}", tag="psadj")
                for k in range(0, KT, 2):
                    nc.tensor.matmul(pt[:, :],
                                     xT8_sb[:, k:k + 2, P * t:P * (t + 1)],
                                     xwT8_sb[:, k:k + 2, :],
                                     start=(k == 0), stop=(k == KT - 2),
                                     perf_mode=DR)
                # mask split DVE (not_equal) 2 : ACT (sign^2) 1
                if t % 3 == 2:
                    nc.scalar.sign(mask_sb[t][:, :], pt[:, :])
                    nc.scalar.square(mask_sb[t][:, :], mask_sb[t][:, :])
                else:
                    nc.vector.tensor_scalar(mask_sb[t][:, :], pt[:, :], 0.0, None,
                                            mybir.AluOpType.not_equal)
                nc.tensor.matmul(pdeg[:, :], ones_col[:, :], mask_sb[t][:, :],
                                 start=(t == 0), stop=(t == NT - 1))

            deg_own = sb.tile([1, R], F32, name="deg_own", tag="deg_own")
            nc.vector.tensor_copy(deg_own[:, :], pdeg[:, :])
            # sqrt(deg) row-vector: cancels the dis_r row scaling for the bias.
            invdis_row = sb.tile([1, R], BF16, name="invdis_row", tag="invdis_row")
            nc.scalar.sqrt(invdis_row[:, :], deg_own[:, :])

            # ---- the ONE collective: AllGather deg (16KB) ------------------
            degb_in = dram.tile([R], F32, name="degb_in", tag="degb_in")
            degb_out = dram.tile([N], F32, addr_space="Shared", name="degb_out", tag="degb_out")
            # row-linear payload: degb_in[r] = deg(own row r); contiguous write
            nc.gpsimd.dma_start(degb_in.rearrange("(m p) -> m p", p=P), deg_own[:, :])
            nc.gpsimd.collective_compute(
                "AllGather", mybir.AluOpType.bypass, replica_groups=rg,
                ins=[degb_in.opt()], outs=[degb_out.opt()])

            # readbacks ride SYNC (clear by then, and its completion
            # signaling is ~5us faster than gpsimd's)
            deg_ownp = sb.tile([P, MT], F32, name="deg_ownp", tag="deg_ownp")
            nc.sync.dma_start(deg_ownp[:, :], degb_in.rearrange("(m p) -> p m", p=P))
            deg_glob = sb.tile([P, NT], F32, name="deg_glob", tag="deg_glob")
            nc.sync.dma_start(
                deg_glob[:, :].rearrange("p (i m) -> p i m", i=N_CORES),
                degb_out.rearrange("(i m p) -> p i m", i=N_CORES, p=P))

            dis_own = sb.tile([P, MT], F32, name="dis_own", tag="dis_own")
            nc.vector.reciprocal(dis_own[:, :], deg_ownp[:, :])
            nc.scalar.sqrt(dis_own[:, :], dis_own[:, :])

            # fence: rotate the ps_a pool with tiny matmuls gated on the deg
            # result, so the y matmuls (and transitively their casts) CANNOT
            # be scheduled into the mask window — masks own DVE/ACT until
            # the collective trigger has fired, y fills the wait afterwards
            for s in range(3):
                pd = ps_a.tile([1, 16], F32, name=f"yfence{s}", tag="psa")
                nc.tensor.matmul(pd[:, :], invdis_row[:, 0:1], invdis_row[:, 0:16],
                                 start=True, stop=True)

            # ---- full y = x @ W_gcn for ALL nodes (local, no collective) ---
            # k-order (1,0,3,2) matches the xTbf arrival order across rings
            YK = (1, 0, 3, 2)
            y_sb = [sb.tile([P, C], BF16, name=f"y{t}", tag=f"y{t}") for t in range(NT)]
            for t in range(NT):
                pa = ps_a.tile([P, C], F32, name=f"psy{t}", tag="psa")
                for ki, k in enumerate(YK):
                    nc.tensor.matmul(pa[:, :],
                                     xTbf_sb[k][:, P * t:P * (t + 1)],
                                     gcnW_sb[k][:, :],
                                     start=(ki == 0), stop=(ki == KT - 1))
                if t % 3 == 2:
                    nc.scalar.copy(y_sb[t][:, :], pa[:, :])
                else:
                    nc.vector.tensor_copy(y_sb[t][:, :], pa[:, :])

            # dis = deg^-1/2 (global, post-AllGather)
            dis_glob = sb.tile([P, NT], F32, name="dis_glob", tag="dis_glob")
            nc.vector.reciprocal(dis_glob[:, :], deg_glob[:, :])
            nc.scalar.sqrt(dis_glob[:, :], dis_glob[:, :])

            # tensor warm-up gated on the gathered deg: ramps the PE p-state
            # during the dis/scale latency window so phase 3 starts hot
            psw = ps_deg.tile([1, R], F32, name="psw", tag="pdeg")
            nc.tensor.matmul(psw[:, 0:NT], ones_f32[:, :], deg_glob[:, :],
                             start=True, stop=True)
            for w in range(6):
                nc.tensor.matmul(psw[:, :], ones_col[:, :], mask_sb[NT - 1][:, :],
                                 start=True, stop=True)

            # ---- phase 3: y *= dis; out_rows = dis_r * (A @ y) + b ----------
            for t in range(NT):
                if t % 3 == 2:
                    nc.scalar.mul(y_sb[t][:, :], y_sb[t][:, :], dis_glob[:, t:t + 1])
                else:
                    nc.vector.tensor_scalar(y_sb[t][:, :], y_sb[t][:, :],
                                            dis_glob[:, t:t + 1], None,
                                            mybir.AluOpType.mult)

            # m-outer: each PSUM bank accumulates a long 33-matmul chain
            for m in range(MT):
                pf = ps_fin.tile([P, C], F32, name=f"psf{m}", tag="psf")
                for t in range(NT):
                    nc.tensor.matmul(pf[:, :],
                                     mask_sb[t][:, P * m:P * (m + 1)],
                                     y_sb[t][:, :],
                                     start=(t == 0), stop=False)
                # += sqrt(deg_r) (x) bias  — cancels against the dis_r scaling
                nc.tensor.matmul(pf[:, :],
                                 invdis_row[:, P * m:P * (m + 1)],
                                 bias_sb[:, :],
                                 start=False, stop=True)
                ot = sbo.tile([P, C], F32, name=f"outt{m}", tag="outt")
                nc.vector.tensor_scalar(ot[:, :], pf[:, :], dis_own[:, m:m + 1],
                                        None, mybir.AluOpType.mult)
                eng = nc.sync if m % 2 == 0 else nc.scalar
                eng.dma_start(out[P * m:P * (m + 1), :], ot[:, :])

    nc.compile()
    return nc


def _get_nc():
    if "nc" not in _cache:
        _cache["nc"] = _build()
    return _cache["nc"]


def _run(inputs, trace=False, trace_cores=None):
    x = np.asarray(inputs["x"], dtype=np.float32)
    adj_weight = np.asarray(inputs["adj_weight"], dtype=np.float32)
    gcn_weight = np.asarray(inputs["gcn_weight"], dtype=np.float32)
    gcn_bias = np.asarray(inputs["gcn_bias"], dtype=np.float32)

    xT = np.ascontiguousarray(x.T)                     # [C, N] f32
    xT8 = xT.astype(F8NP)
    xTbf = xT.astype(BF)
    adjW8 = adj_weight.astype(F8NP)
    gcnW = gcn_weight.astype(BF)
    bias_bf = gcn_bias.reshape(1, C).astype(BF)

    in_maps = []
    for i in range(N_CORES):
        in_maps.append({
            "xT8": xT8,
            "xTs8": np.ascontiguousarray(xT8[:, R * i:R * (i + 1)]),
            "adjW8": adjW8,
            "xTbf": xTbf,
            "gcnW": gcnW,
            "bias": bias_bf,
        })

    nc = _get_nc()
    res = run_bass_kernel_spmd(nc, in_maps, core_ids=list(range(N_CORES)),
                               trace=trace, trace_cores=trace_cores)
    full = np.concatenate([res.results[i]["out"] for i in range(N_CORES)], axis=0)
    return full, res


def kernel(**inputs):
    full, _ = _run(inputs, trace=False)
    return full


# revision 18
# speedup vs baseline: 1.0841x; 1.0621x over previous
"""Distributed Trainium2 (Bass/Tile) kernel for AdaptiveGCNLayer.

Reference semantics (N=4096 nodes, C=512 channels):
    adj   = x @ W_adj @ x.T + I                      [N, N]
    adj   = d^-1/2 * adj * d^-1/2   (row sums d)     -- values then DISCARDED:
    A     = (adj != 0) with forced unit diagonal     (dense_to_sparse keeps only
                                                      the nonzero pattern)
    deg   = A.sum(1); dis = deg^-1/2 (0 if deg<=0)
    out   = (dis[:,None] * A * dis[None,:]) @ (x @ W_gcn) + b

Scaling rows/cols by nonzero (or NaN/inf) factors never changes the !=0
pattern, so A == (x @ W_adj @ x.T != 0) except on the measure-zero event of
an exactly-zero f32 entry; the first normalization is therefore not
materialized, and the adjacency can be computed at any precision (fp8 here)
since only its zero pattern survives.

Sharding (8 cores, 1-D node partition, R=512 rows each): core i computes its
adjacency block in TRANSPOSED layout adjT [N, R] (directly usable as the
stationary operand of the final aggregation), masks it to {0,1} bf16, and
reduces mask -> deg for its rows (ones-matmul chain on the TensorEngine).

Collective structure: this environment has a ~40-46us rank-dispatch skew —
the mesh for ANY collective begins only once the LAST core reaches its
trigger, so the measured core-0 span is
  skew + slowest-core time-to-trigger + mesh + post-collective work.
There is exactly ONE collective: the 16KB deg AllGather (~8us mesh).
y = x @ W_gcn is computed locally IN FULL on every core (~30us of
redundant TensorE work) inside the otherwise-dead window while waiting
for the deg exchange — redundant FLOPs are free there, wire time is not.
Critical path: loads -> xwT -> adjacency+mask+deg (~30us local) -> deg
AllGather -> dis -> aggregation.

Scheduling notes (hard-won from traces):
  - ~7.3us fixed engine-bringup preamble before any user work
  - each DMA ring delivers ~100GB/s serialized in ring order, so ring
    POSITION is arrival time; the critical bytes (1a operands, then xT8)
    sit at the FRONT of the sync+scalar rings, split between them
  - xT8 is loaded in half-column chunks so adjacency tiles 0-15 can
    start ~2.5us before the full tensor lands
  - the Tile scheduler reorders per-engine instructions by readiness, so
    program order alone cannot keep y-phase DVE/ACT work (casts) out of
    the mask window; instead xTbf/gcnW are placed ring-LATE so y matmuls
    physically cannot start before the deg trigger has fired
  - gpsimd's ring carries ONLY the deg bounce + AllGather + readbacks
    (gpsimd-ring completion signaling is slow; nothing critical rides it)
  - mask computation is split DVE(not_equal) 2 : ACT(sign^2) 1, keeping
    both engines under the phase-2 wall; adjacency runs fp8 DoubleRow
  - the deg payload is written row-linear; readbacks pay the strided
    transpose (16KB, ~1us)
  - the bias enters through a rank-1 matmul sqrt(deg_r) (x) bias folded
    into the aggregation PSUM (cancels the later dis_r row scaling)
"""

import numpy as np

from concourse import bacc, mybir, tile
from concourse.bass_utils import run_bass_kernel_spmd

N_CORES = 8
N = 4096               # nodes
C = 512                # channels (C_IN == C_OUT)
R = N // N_CORES       # 512 rows per core
P = 128                # SBUF partitions
KT = C // P            # 4 contraction tiles
NT = N // P            # 32 node tiles
MT = R // P            # 4 row tiles per core
H = N // 2             # xT8 half-chunk columns

F32 = mybir.dt.float32
BF16 = mybir.dt.bfloat16
F8 = mybir.dt.float8e4
BF = mybir.dt.np(BF16)
F8NP = mybir.dt.np(F8)
DR = mybir.MatmulPerfMode.DoubleRow

_cache = {}


def _build():
    nc = bacc.Bacc("TRN2", target_bir_lowering=False, debug=False,
                   num_devices=N_CORES)

    xT8 = nc.dram_tensor("xT8", [C, N], F8, kind="ExternalInput")      # x^T, full
    xTs8 = nc.dram_tensor("xTs8", [C, R], F8, kind="ExternalInput")    # own cols
    adjW8 = nc.dram_tensor("adjW8", [C, C], F8, kind="ExternalInput")
    xTbf = nc.dram_tensor("xTbf", [C, N], BF16, kind="ExternalInput")  # x^T, full
    gcnW = nc.dram_tensor("gcnW", [C, C], BF16, kind="ExternalInput")
    bias = nc.dram_tensor("bias", [1, C], BF16, kind="ExternalInput")
    out = nc.dram_tensor("out", [R, C], F32, kind="ExternalOutput")

    rg = [list(range(N_CORES))]

    with tile.TileContext(nc) as tc:
        with (
            tc.tile_pool(name="sb", bufs=1) as sb,
            tc.tile_pool(name="sbo", bufs=2) as sbo,
            tc.tile_pool(name="dram", bufs=1, space="DRAM") as dram,
            tc.tile_pool(name="ps_a", bufs=2, space="PSUM") as ps_a,
            tc.tile_pool(name="ps_adj", bufs=3, space="PSUM") as ps_adj,
            tc.tile_pool(name="ps_deg", bufs=1, space="PSUM") as ps_deg,
            tc.tile_pool(name="ps_fin", bufs=2, space="PSUM") as ps_fin,
        ):
            # ---- SBUF tiles -------------------------------------------------
            adjW8_sb = sb.tile([P, KT, C], F8, name="adjW8_sb", tag="adjW8_sb")
            xTs8_sb = sb.tile([P, KT, R], F8, name="xTs8_sb", tag="xTs8_sb")
            xT8_sb = sb.tile([P, KT, N], F8, name="xT8_sb", tag="xT8_sb")
            xTbf_sb = [sb.tile([P, N], BF16, name=f"xTbf{k}", tag=f"xTbf{k}") for k in range(KT)]
            gcnW_sb = [sb.tile([P, C], BF16, name=f"gcnW{k}", tag=f"gcnW{k}") for k in range(KT)]
            bias_sb = sb.tile([1, C], BF16, name="bias_sb", tag="bias_sb")
            ones_col = sb.tile([P, 1], BF16, name="ones_col", tag="ones_col")
            ones_f32 = sb.tile([P, 1], F32, name="ones_f32", tag="ones_f32")
            scr = sb.tile([1, 8], F32, name="scr", tag="scr")

            # ---- input loads: ring position == arrival time ---------------
            # The ACT engine's compute shares its sequencer with the scalar
            # DMA ring, so the scalar ring carries ONLY the critical bytes
            # (1a operands + xT8) plus two ring-late xTbf slices; everything
            # else rides sync.
            nc.sync.dma_start(adjW8_sb[:, 0, :], adjW8[0:P, :])
            nc.sync.dma_start(adjW8_sb[:, 1, :], adjW8[P:2 * P, :])
            nc.sync.dma_start(xTs8_sb[:, 0, :], xTs8[0:P, :])
            nc.sync.dma_start(xTs8_sb[:, 1, :], xTs8[P:2 * P, :])
            nc.scalar.dma_start(adjW8_sb[:, 2, :], adjW8[2 * P:3 * P, :])
            nc.scalar.dma_start(adjW8_sb[:, 3, :], adjW8[3 * P:4 * P, :])
            nc.scalar.dma_start(xTs8_sb[:, 2, :], xTs8[2 * P:3 * P, :])
            nc.scalar.dma_start(xTs8_sb[:, 3, :], xTs8[3 * P:4 * P, :])
            for h in range(2):
                nc.sync.dma_start(xT8_sb[:, 0, H * h:H * (h + 1)], xT8[0:P, H * h:H * (h + 1)])
                nc.sync.dma_start(xT8_sb[:, 1, H * h:H * (h + 1)], xT8[P:2 * P, H * h:H * (h + 1)])
                nc.scalar.dma_start(xT8_sb[:, 2, H * h:H * (h + 1)], xT8[2 * P:3 * P, H * h:H * (h + 1)])
                nc.scalar.dma_start(xT8_sb[:, 3, H * h:H * (h + 1)], xT8[3 * P:4 * P, H * h:H * (h + 1)])
            nc.sync.dma_start(bias_sb[:, :], bias[:, :])
            for k in range(KT):
                nc.sync.dma_start(gcnW_sb[k][:, :], gcnW[P * k:P * (k + 1), :])
            # y stationary ring-LATE: y matmuls start only as these land, so
            # their casts cannot crowd the mask window; the y accumulation
            # consumes the k-slices in arrival order (1,0,3,2 across rings)
            nc.sync.dma_start(xTbf_sb[0][:, :], xTbf[0:P, :])
            nc.scalar.dma_start(xTbf_sb[1][:, :], xTbf[P:2 * P, :])
            nc.sync.dma_start(xTbf_sb[2][:, :], xTbf[2 * P:3 * P, :])
            nc.scalar.dma_start(xTbf_sb[3][:, :], xTbf[3 * P:4 * P, :])

            nc.vector.memset(ones_col[:, :], 1.0)
            nc.vector.memset(ones_f32[:, :], 1.0)
            # preload DVE reciprocal / ACT sqrt+sign lookup tables off the
            # critical path (first use otherwise costs ~1.3us each)
            nc.vector.memset(scr[:, 0:4], 4.0)
            nc.vector.reciprocal(scr[:, 4:8], scr[:, 0:4])
            nc.scalar.sqrt(scr[:, 4:8], scr[:, 0:4])
            nc.scalar.sign(scr[:, 4:8], scr[:, 0:4])

            # ---- phase 1a: xwT[j, r] = sum_c W_adj[c, j] x^T[c, r]  (fp8 DR)
            xwT8_sb = sb.tile([P, KT, R], F8, name="xwT8_sb", tag="xwT8_sb")
            for j in range(KT):
                pa = ps_a.tile([P, R], F32, name=f"psa{j}", tag="psa")
                for k in range(0, KT, 2):
                    nc.tensor.matmul(pa[:, :],
                                     adjW8_sb[:, k:k + 2, P * j:P * (j + 1)],
                                     xTs8_sb[:, k:k + 2, :],
                                     start=(k == 0), stop=(k == KT - 2),
                                     perf_mode=DR)
                if j < 2:
                    nc.vector.tensor_copy(xwT8_sb[:, j, :], pa[:, :])
                else:
                    nc.scalar.copy(xwT8_sb[:, j, :], pa[:, :])

            # ---- phase 2: adjT tiles (fp8 DR), mask (bf16), deg ------------
            mask_sb = [sb.tile([P, R], BF16, name=f"mask{t}", tag=f"mask{t}") for t in range(NT)]
            pdeg = ps_deg.tile([1, R], F32, name="pdeg", tag="pdeg")
            for t in range(NT):
                pt = ps_adj.tile([P, R], F32, name=f"psadj{t}", tag="psadj")
                for k in range(0, KT, 2):
                    nc.tensor.matmul(pt[:, :],
                                     xT8_sb[:, k:k + 2, P * t:P * (t + 1)],
                                     xwT8_sb[:, k:k + 2, :],
                                     start=(k == 0), stop=(k == KT - 2),
                                     perf_mode=DR)
                # mask split DVE (not_equal) 2 : ACT (sign^2) 1
                if t % 3 == 2:
                    nc.scalar.sign(mask_sb[t][:, :], pt[:, :])
                    nc.scalar.square(mask_sb[t][:, :], mask_sb[t][:, :])
                else:
                    nc.vector.tensor_scalar(mask_sb[t][:, :], pt[:, :], 0.0, None,
                                            mybir.AluOpType.not_equal)
                nc.tensor.matmul(pdeg[:, :], ones_col[:, :], mask_sb[t][:, :],
                                 start=(t == 0), stop=(t == NT - 1))
                # mask-gated filler: keeps the PE continuously busy so its
                # p-state stays ramped while masks pace the pipeline (an
                # idle-stalled PE drops to ~2x slower matmuls)
                if 2 <= t < 16:
                    pw = ps_fin.tile([1, 256], F32, name=f"fill{t}", tag="psf")
                    nc.tensor.matmul(pw[:, :], ones_col[:, :],
                                     mask_sb[t - 2][:, 0:256],
                                     start=True, stop=True)

            deg_own = sb.tile([1, R], F32, name="deg_own", tag="deg_own")
            nc.vector.tensor_copy(deg_own[:, :], pdeg[:, :])
            # gate tile written only after deg: the y casts multiply by this
            # 1.0, so they cannot be scheduled into the mask window
            gate1 = sb.tile([P, 1], F32, name="gate1", tag="gate1")
            nc.vector.memset(gate1[:, :], 1.0)
            # sqrt(deg) row-vector: cancels the dis_r row scaling for the bias.
            invdis_row = sb.tile([1, R], BF16, name="invdis_row", tag="invdis_row")
            nc.scalar.sqrt(invdis_row[:, :], deg_own[:, :])

            # ---- the ONE collective: AllGather deg (16KB) ------------------
            degb_in = dram.tile([R], F32, name="degb_in", tag="degb_in")
            degb_out = dram.tile([N], F32, addr_space="Shared", name="degb_out", tag="degb_out")
            # row-linear payload: degb_in[r] = deg(own row r); contiguous write
            nc.gpsimd.dma_start(degb_in.rearrange("(m p) -> m p", p=P), deg_own[:, :])
            nc.gpsimd.collective_compute(
                "AllGather", mybir.AluOpType.bypass, replica_groups=rg,
                ins=[degb_in.opt()], outs=[degb_out.opt()])

            # readbacks ride SYNC (clear by then, and its completion
            # signaling is ~5us faster than gpsimd's)
            deg_ownp = sb.tile([P, MT], F32, name="deg_ownp", tag="deg_ownp")
            nc.sync.dma_start(deg_ownp[:, :], degb_in.rearrange("(m p) -> p m", p=P))
            deg_glob = sb.tile([P, NT], F32, name="deg_glob", tag="deg_glob")
            nc.sync.dma_start(
                deg_glob[:, :].rearrange("p (i m) -> p i m", i=N_CORES),
                degb_out.rearrange("(i m p) -> p i m", i=N_CORES, p=P))

            dis_own = sb.tile([P, MT], F32, name="dis_own", tag="dis_own")
            nc.vector.reciprocal(dis_own[:, :], deg_ownp[:, :])
            nc.scalar.sqrt(dis_own[:, :], dis_own[:, :])

            # ---- full y = x @ W_gcn for ALL nodes (local, no collective) ---
            # y matmuls run free (they are the tensor filler that keeps the
            # p-state ramped); the CASTS are gated on gate1 (post-deg) so
            # DVE/ACT stay exclusive to masks until the trigger fires.
            # k-order (1,0,3,2) matches the xTbf arrival order across rings
            YK = (1, 0, 3, 2)
            y_sb = [sb.tile([P, C], BF16, name=f"y{t}", tag=f"y{t}") for t in range(NT)]
            for t in range(NT):
                pa = ps_a.tile([P, C], F32, name=f"psy{t}", tag="psa")
                for ki, k in enumerate(YK):
                    nc.tensor.matmul(pa[:, :],
                                     xTbf_sb[k][:, P * t:P * (t + 1)],
                                     gcnW_sb[k][:, :],
                                     start=(ki == 0), stop=(ki == KT - 1))
                if t % 3 == 2:
                    nc.scalar.mul(y_sb[t][:, :], pa[:, :], gate1[:, 0:1])
                else:
                    nc.vector.tensor_scalar(y_sb[t][:, :], pa[:, :],
                                            gate1[:, 0:1], None,
                                            mybir.AluOpType.mult)

            # dis = deg^-1/2 (global, post-AllGather)
            dis_glob = sb.tile([P, NT], F32, name="dis_glob", tag="dis_glob")
            nc.vector.reciprocal(dis_glob[:, :], deg_glob[:, :])
            nc.scalar.sqrt(dis_glob[:, :], dis_glob[:, :])

            # tensor warm-up gated on the gathered deg: ramps the PE p-state
            # during the dis/scale latency window so phase 3 starts hot
            psw = ps_deg.tile([1, R], F32, name="psw", tag="pdeg")
            nc.tensor.matmul(psw[:, 0:NT], ones_f32[:, :], deg_glob[:, :],
                             start=True, stop=True)
            for w in range(6):
                nc.tensor.matmul(psw[:, :], ones_col[:, :], mask_sb[NT - 1][:, :],
                                 start=True, stop=True)

            # ---- phase 3: y *= dis; out_rows = dis_r * (A @ y) + b ----------
            for t in range(NT):
                if t % 3 == 2:
                    nc.scalar.mul(y_sb[t][:, :], y_sb[t][:, :], dis_glob[:, t:t + 1])
                else:
                    nc.vector.tensor_scalar(y_sb[t][:, :], y_sb[t][:, :],
                                            dis_glob[:, t:t + 1], None,
                                            mybir.AluOpType.mult)

            # m-outer: each PSUM bank accumulates a long 33-matmul chain
            for m in range(MT):
                pf = ps_fin.tile([P, C], F32, name=f"psf{m}", tag="psf")
                for t in range(NT):
                    nc.tensor.matmul(pf[:, :],
                                     mask_sb[t][:, P * m:P * (m + 1)],
                                     y_sb[t][:, :],
                                     start=(t == 0), stop=False)
                # += sqrt(deg_r) (x) bias  — cancels against the dis_r scaling
                nc.tensor.matmul(pf[:, :],
                                 invdis_row[:, P * m:P * (m + 1)],
                                 bias_sb[:, :],
                                 start=False, stop=True)
                ot = sbo.tile([P, C], F32, name=f"outt{m}", tag="outt")
                nc.vector.tensor_scalar(ot[:, :], pf[:, :], dis_own[:, m:m + 1],
                                        None, mybir.AluOpType.mult)
                eng = nc.sync if m % 2 == 0 else nc.scalar
                eng.dma_start(out[P * m:P * (m + 1), :], ot[:, :])

    nc.compile()
    return nc


def _get_nc():
    if "nc" not in _cache:
        _cache["nc"] = _build()
    return _cache["nc"]


def _run(inputs, trace=False, trace_cores=None):
    x = np.asarray(inputs["x"], dtype=np.float32)
    adj_weight = np.asarray(inputs["adj_weight"], dtype=np.float32)
    gcn_weight = np.asarray(inputs["gcn_weight"], dtype=np.float32)
    gcn_bias = np.asarray(inputs["gcn_bias"], dtype=np.float32)

    xT = np.ascontiguousarray(x.T)                     # [C, N] f32
    xT8 = xT.astype(F8NP)
    xTbf = xT.astype(BF)
    adjW8 = adj_weight.astype(F8NP)
    gcnW = gcn_weight.astype(BF)
    bias_bf = gcn_bias.reshape(1, C).astype(BF)

    in_maps = []
    for i in range(N_CORES):
        in_maps.append({
            "xT8": xT8,
            "xTs8": np.ascontiguousarray(xT8[:, R * i:R * (i + 1)]),
            "adjW8": adjW8,
            "xTbf": xTbf,
            "gcnW": gcnW,
            "bias": bias_bf,
        })

    nc = _get_nc()
    res = run_bass_kernel_spmd(nc, in_maps, core_ids=list(range(N_CORES)),
                               trace=trace, trace_cores=trace_cores)
    full = np.concatenate([res.results[i]["out"] for i in range(N_CORES)], axis=0)
    return full, res


def kernel(**inputs):
    full, _ = _run(inputs, trace=False)
    return full
